# revision 1
# baseline (speedup 1.0000x reference)
"""Trainium2 Bass kernel for nn_AttentionBlock (B=16, C=512, H=W=32, 8 heads).

Sharding: data-parallel over batch across 8 NeuronCores (2 batch elems/core).
No collectives: each core runs the same NEFF on its own batch slice.

Per-core algorithm (per batch element), all layouts chosen so no transposes
are ever needed:
  x_b is [C=512, S=1024] in natural layout (C on partitions, k-tiled by 128).
  Phase 1 (QKV):
    q/k:   psum[128, S] = [Wq_h | Wk_h]^T @ x  (head h's q,k columns are
           contiguous in w_qkv) -> q_h^T on psum partitions 0-63, k_h^T on
           64-127; copied to SBUF with heads 2a/2a+1 packed on partition
           halves so a pair's score matmuls can co-run on PE row halves.
    v:     out = x^T @ W_v      ->  [S, 8*64] natural layout (S on partitions),
           stored bf16 with a constant ones column appended per head ([.., 65]).
  Phase 2 (attention, per head):
    scores^T = kT.T @ qT  -> psum [128 j, 1024 i]   (K=64; heads of a pair run
               concurrently on PE row-halves 0-63 / 64-127 via tile_position)
    p^T = exp(scores^T * 0.125)  (ScalarE, no max subtraction: |s|<~8, safe)
    [out^T | rowsum] = [v | 1]^T @ p^T  -> psum [65, 1024 i] accum over j tiles
    o^T = out^T * bcast(1/rowsum)  (reciprocal on DVE; broadcast over the 64
          partitions via a K=1 matmul with a ones vector)
  Phase 3: y^T = W_p^T @ o^T + b + x  -> [C, S] natural; DMA out.

Matmuls run as float32r (fp32 data rounded by DVE, single 'High' weight pass,
~4x faster than full fp32) except the P@V stage which is bf16 (p is in [0,1]
scale and v is already truncated by the f32r weight path anyway).
"""

import numpy as np

import concourse.bacc as bacc
import concourse.bass as bass
import concourse.mybir as mybir
import concourse.tile as tile

F32 = mybir.dt.float32
BF16 = mybir.dt.bfloat16
F32R = mybir.dt.float32r

B, C, HW, NH, DK = 16, 512, 1024, 8, 64
NCORES = 8
BPC = B // NCORES          # batch elems per core
P = 128
KT = C // P                # 4 contraction tiles over C
NPAIR = NH // 2            # 4 head pairs
SC = HW // 512             # 2 s-chunks of 512
ST = HW // P               # 8 s-tiles of 128 (j tiles)
SCALE = DK ** -0.5

# 'bf16' | 'f32r' | 'f32'  (f32r: more accurate but serialized weight loads;
#  f32: exact but ~4x slower matmuls. 'f32r'/'f32' need smaller pool bufs to fit)
MM_DTYPE = "bf16"
# P@V dtype: bf16 (fast, saves SBUF)
PV_BF16 = True


def build_program(with_bias: bool, mm_dtype: str = MM_DTYPE):
    MMDT = {"f32r": F32R, "bf16": BF16, "f32": F32}[mm_dtype]
    nc = bacc.Bacc(None, target_bir_lowering=False, debug=False)

    x_d = nc.dram_tensor("x", [BPC, C, HW], F32, kind="ExternalInput")
    wqkv_d = nc.dram_tensor("w_qkv", [C, 3 * C], F32, kind="ExternalInput")
    wproj_d = nc.dram_tensor("w_proj", [C, C], F32, kind="ExternalInput")
    if with_bias:
        bqkv_d = nc.dram_tensor("b_qkv", [3 * C], F32, kind="ExternalInput")
        bproj_d = nc.dram_tensor("b_proj", [C], F32, kind="ExternalInput")
    out_d = nc.dram_tensor("out", [BPC, C, HW], F32, kind="ExternalOutput")

    pv_dt = BF16 if PV_BF16 else MMDT

    with tile.TileContext(nc) as tc:
        with tc.tile_pool(name="consts", bufs=1) as consts:
            # Persistent weight buffers (f32r-rounded by DVE as required for
            # fp32r matmul operands).
            wqk_r = consts.tile([P, KT, NH, 2 * DK], MMDT)
            wv_sb = consts.tile([P, KT, C], MMDT)
            wproj_r = consts.tile([P, KT, C], MMDT)
            ones_f32 = consts.tile([1, P], F32)
            nc.vector.memset(ones_f32, 1.0)
            ones_sb = consts.tile([1, P], MMDT)
            nc.vector.tensor_copy(out=ones_sb, in_=ones_f32)

            if with_bias:
                bq_sb = consts.tile([P, NPAIR], F32)
                bk_sb = consts.tile([P, NPAIR], F32)
                bv_sb = consts.tile([1, C], MMDT)
                bp_sb = consts.tile([P, KT], F32)

            # Staging pool: opened after consts, closed before the main pools
            # so its SBUF is reclaimed for the batch working set.
            with tc.tile_pool(name="wstage", bufs=1) as wstagep:
                wq_st = wstagep.tile([P, KT, 3 * C], F32)
                nc.sync.dma_start(
                    out=wq_st, in_=wqkv_d[:].rearrange("(kt p) n -> p kt n", p=P)
                )
                ws4 = wq_st.rearrange("p kt (h t) -> p kt h t", t=3 * DK)
                for kt in range(KT):
                    nc.vector.tensor_copy(
                        out=wqk_r[:, kt], in_=ws4[:, kt, :, 0 : 2 * DK]
                    )
                    nc.vector.tensor_copy(
                        out=wv_sb[:, kt, :].rearrange("p (h t) -> p h t", t=DK),
                        in_=ws4[:, kt, :, 2 * DK :],
                    )
                wp_st = wstagep.tile([P, KT, C], F32)
                nc.sync.dma_start(
                    out=wp_st, in_=wproj_d[:].rearrange("(kt p) n -> p kt n", p=P)
                )
                nc.vector.tensor_copy(
                    out=wproj_r.rearrange("p kt n -> p (kt n)"),
                    in_=wp_st.rearrange("p kt n -> p (kt n)"),
                )
                if with_bias:
                    b3 = bqkv_d[:].rearrange("(h t) -> h t", t=3 * DK)  # [8,192]
                    for m in range(2):
                        # heads m::2 -> partitions m*64.. ; free dim = pair idx
                        nc.sync.dma_start(
                            out=bq_sb[m * DK : (m + 1) * DK, :],
                            in_=b3[m::2, 0:DK].rearrange("a t -> t a"),
                        )
                        nc.sync.dma_start(
                            out=bk_sb[m * DK : (m + 1) * DK, :],
                            in_=b3[m::2, DK : 2 * DK].rearrange("a t -> t a"),
                        )
                    bv_st = wstagep.tile([1, C], F32)
                    nc.sync.dma_start(
                        out=bv_st,
                        in_=b3[:, 2 * DK :].rearrange("h t -> (h t)")[None, :],
                    )
                    nc.vector.tensor_copy(out=bv_sb, in_=bv_st)
                    nc.sync.dma_start(
                        out=bp_sb, in_=bproj_d[:].rearrange("(a p) -> p a", p=P)
                    )

            # Funnel all staging deps through one barrier so the batch-loop
            # DMAs don't inherit a multi-semaphore wait set (HW DMA
            # instructions have very few wait slots).
            tc.strict_bb_all_engine_barrier()

            with (
                tc.tile_pool(name="xp", bufs=1) as xp,
                tc.tile_pool(name="qkt", bufs=1) as qktp,
                tc.tile_pool(name="vp", bufs=1) as vp,
                tc.tile_pool(name="pt", bufs=3) as ptp,
                tc.tile_pool(name="ot", bufs=1) as otp,
                tc.tile_pool(name="stage", bufs=1) as stagep,
                tc.tile_pool(name="rc", bufs=2) as rcp,
                tc.tile_pool(name="rcb", bufs=2) as rcbp,
                tc.tile_pool(name="psS", bufs=2, space="PSUM") as psS,
                tc.tile_pool(name="psV", bufs=2, space="PSUM") as psV,
            ):
                for b in range(BPC):
                    # ---- load x_b as [P, KT, S]; keep exact f32 for the
                    # residual plus a DVE-rounded f32r copy for matmuls.
                    x_t = xp.tile([P, KT, HW], F32, tag="x", name=f"x{b}")
                    nc.sync.dma_start(
                        out=x_t, in_=x_d[b].rearrange("(kt p) s -> p kt s", p=P)
                    )
                    x_r = xp.tile([P, KT, HW], MMDT, tag="xr", name=f"xr{b}")
                    nc.vector.tensor_copy(
                        out=x_r.rearrange("p kt s -> p (kt s)"),
                        in_=x_t.rearrange("p kt s -> p (kt s)"),
                    )
                    # residual: pre-copy x into the output in DRAM; the proj
                    # result is DMA-accumulated onto it at the end.
                    nc.sync.dma_start(out=out_d[b], in_=x_d[b])

                    # ---- phase 1: q^T/k^T per head ----
                    # qkT[P, {q,k}, pair, S]: partitions 0-63 head 2a,
                    # 64-127 head 2a+1.
                    qkT = qktp.tile([P, 2, NPAIR, HW], MMDT, tag="qkT",
                                    name=f"qkT{b}", bufs=2)
                    for h in range(NH):
                        a, m = h // 2, h % 2
                        ps = psS.tile([P, 1024], F32, tag="mm", name=f"ps_qk{h}")
                        for sc in range(SC):
                            for kt in range(KT):
                                nc.tensor.matmul(
                                    ps[:, sc * 512 : (sc + 1) * 512],
                                    lhsT=wqk_r[:, kt, h, :],
                                    rhs=x_r[:, kt, sc * 512 : (sc + 1) * 512],
                                    start=(kt == 0),
                                    stop=(kt == KT - 1),
                                )
                        half = slice(m * DK, (m + 1) * DK)
                        for qk in range(2):  # 0=q (psum 0:64), 1=k (psum 64:128)
                            src = ps[qk * DK : (qk + 1) * DK, :]
                            if with_bias:
                                bsb = bq_sb if qk == 0 else bk_sb
                                nc.vector.tensor_scalar(
                                    out=qkT[half, qk, a, :],
                                    in0=src,
                                    scalar1=bsb[half, a : a + 1],
                                    scalar2=None,
                                    op0=mybir.AluOpType.add,
                                )
                            else:
                                nc.vector.tensor_copy(
                                    out=qkT[half, qk, a, :], in_=src
                                )

                    # ---- phase 1: v natural [P(s), st, head, 65], ones col ----
                    v_sb = vp.tile([P, ST, NH, DK + 1], pv_dt, tag="v",
                                   name=f"v{b}")
                    nc.vector.memset(
                        v_sb.rearrange("p st h t -> p (st h) t")[:, :, DK:], 1.0
                    )
                    for mt2 in range(ST // 2):
                        ps = psS.tile([P, 1024], F32, tag="mm", name=f"ps_v{mt2}")
                        for half_i in range(2):
                            mt = 2 * mt2 + half_i
                            for kt in range(KT):
                                nc.tensor.matmul(
                                    ps[:, half_i * 512 : (half_i + 1) * 512],
                                    lhsT=x_r[:, kt, mt * P : (mt + 1) * P],
                                    rhs=wv_sb[:, kt, :],
                                    start=(kt == 0),
                                    stop=(kt == KT - 1) if not with_bias else False,
                                )
                            if with_bias:
                                # += ones^T @ b_v (adds b_v to every row)
                                nc.tensor.matmul(
                                    ps[:, half_i * 512 : (half_i + 1) * 512],
                                    lhsT=ones_sb,
                                    rhs=bv_sb,
                                    start=False,
                                    stop=True,
                                )
                        for half_i in range(2):
                            mt = 2 * mt2 + half_i
                            nc.vector.tensor_copy(
                                out=v_sb[:, mt, :, 0:DK],
                                in_=ps[:, half_i * 512 : (half_i + 1) * 512]
                                .rearrange("p (h t) -> p h t", h=NH),
                            )

                    # ---- phase 2: attention, software-pipelined pairs ----
                    # Per jt step: 4 score MMs of pair a interleave with the 4
                    # accumulating P@V MMs of pair a-1, so PE stays dense while
                    # ScalarE paces the exps. PSUM: 2 score tiles (4 banks) +
                    # 4 PV chains (4 banks).
                    oT = otp.tile([P, NPAIR, HW], MMDT, tag="oT", name=f"oT{b}")

                    def pv_step(a_p, pts_p, pvs_p, jt):
                        for m in range(2):
                            h = 2 * a_p + m
                            for sc in range(SC):
                                nc.tensor.matmul(
                                    pvs_p[m][sc],
                                    lhsT=v_sb[:, jt, h, :],
                                    rhs=pts_p[m][:, jt, sc * 512 : (sc + 1) * 512],
                                    start=(jt == 0),
                                    stop=(jt == ST - 1),
                                )

                    def pv_finish(a_p, pvs_p):
                        for m in range(2):
                            for sc in range(SC):
                                pv = pvs_p[m][sc]
                                rs = rcp.tile([1, 512], F32, tag="rs", name="rs")
                                nc.vector.tensor_copy(
                                    out=rs, in_=pv[DK : DK + 1, :]
                                )
                                rc = rcp.tile([1, 512], F32, tag="rc", name="rc")
                                nc.vector.reciprocal_approx_fast(out=rc, in_=rs)
                                rcb = rcbp.tile([DK, 512], F32, tag="rcb",
                                                name="rcb")
                                nc.gpsimd.partition_broadcast(rcb, rc)
                                osl = oT[m * DK : (m + 1) * DK, a_p,
                                         sc * 512 : (sc + 1) * 512]
                                nc.vector.tensor_tensor(
                                    out=osl, in0=pv[0:DK, :], in1=rcb,
                                    op=mybir.AluOpType.mult,
                                )

                    prev = None  # (a, pts, pvs)
                    for a in range(NPAIR):
                        pts = [
                            ptp.tile([P, ST, HW], pv_dt, tag="pt",
                                     name=f"pt{a}_{m}", bufs=4)
                            for m in range(2)
                        ]
                        pvs = [
                            [
                                psV.tile([DK + 1, 512], F32, tag="pv",
                                         name=f"pv{a}_{m}_{sc}", bufs=4)
                                for sc in range(SC)
                            ]
                            for m in range(2)
                        ]
                        for jt in range(ST):
                            pss = [
                                psS.tile([P, 1024], F32, tag="mm",
                                         name=f"ps_s{m}")
                                for m in range(2)
                            ]
                            for sc in range(SC):
                                for m in range(2):
                                    lo, hi = m * DK, (m + 1) * DK
                                    nc.tensor.matmul(
                                        pss[m][:, sc * 512 : (sc + 1) * 512],
                                        lhsT=qkT[lo:hi, 1, a,
                                                 jt * P : (jt + 1) * P],
                                        rhs=qkT[lo:hi, 0, a,
                                                sc * 512 : (sc + 1) * 512],
                                        start=True,
                                        stop=True,
                                    )
                            if prev is not None:
                                pv_step(prev[0], prev[1], prev[2], jt)
                            for m in range(2):
                                nc.scalar.activation(
                                    out=pts[m][:, jt, :],
                                    in_=pss[m],
                                    func=mybir.ActivationFunctionType.Exp,
                                    scale=SCALE,
                                )
                        if prev is not None:
                            pv_finish(prev[0], prev[2])
                        prev = (a, pts, pvs)

                    # drain the last pair's P@V
                    for jt in range(ST):
                        pv_step(prev[0], prev[1], prev[2], jt)
                    pv_finish(prev[0], prev[2])

                    # ---- phase 3: proj + bias + residual ----
                    for a in range(KT):
                        ps = psS.tile([P, 1024], F32, tag="mm", name=f"ps_p{a}")
                        for sc in range(SC):
                            for kt in range(KT):
                                nc.tensor.matmul(
                                    ps[:, sc * 512 : (sc + 1) * 512],
                                    lhsT=wproj_r[:, kt, a * P : (a + 1) * P],
                                    rhs=oT[:, kt, sc * 512 : (sc + 1) * 512],
                                    start=(kt == 0),
                                    stop=(kt == KT - 1),
                                )
                        yt = stagep.tile([P, 1024], F32, tag="y", name=f"yt{a}")
                        if with_bias:
                            nc.vector.tensor_scalar(
                                out=yt, in0=ps, scalar1=bp_sb[:, a : a + 1],
                                scalar2=None, op0=mybir.AluOpType.add,
                            )
                        else:
                            nc.vector.tensor_copy(out=yt, in_=ps)
                        nc.gpsimd.dma_start(
                            out=out_d[b].rearrange("(kt p) s -> p kt s", p=P)
                            [:, a, :],
                            in_=yt,
                            accum_op=mybir.AluOpType.add,
                        )

    nc.finalize()
    return nc


_CACHE = {}


def _get_program(with_bias: bool, mm_dtype: str = MM_DTYPE):
    key = (with_bias, mm_dtype)
    if key not in _CACHE:
        _CACHE[key] = build_program(with_bias, mm_dtype)
    return _CACHE[key]


def kernel(x, w_qkv, b_qkv, w_proj, b_proj):
    x = np.ascontiguousarray(np.asarray(x, dtype=np.float32)).reshape(B, C, HW)
    w_qkv = np.ascontiguousarray(np.asarray(w_qkv, dtype=np.float32))
    b_qkv = np.ascontiguousarray(np.asarray(b_qkv, dtype=np.float32))
    w_proj = np.ascontiguousarray(np.asarray(w_proj, dtype=np.float32))
    b_proj = np.ascontiguousarray(np.asarray(b_proj, dtype=np.float32))

    with_bias = bool(np.any(b_qkv) or np.any(b_proj))
    nc = _get_program(with_bias, MM_DTYPE)

    in_maps = []
    for i in range(NCORES):
        m = {
            "x": x[i * BPC : (i + 1) * BPC],
            "w_qkv": w_qkv,
            "w_proj": w_proj,
        }
        if with_bias:
            m["b_qkv"] = b_qkv
            m["b_proj"] = b_proj
        in_maps.append(m)

    from concourse.bass_utils import run_bass_kernel_spmd

    res = run_bass_kernel_spmd(nc, in_maps, core_ids=list(range(NCORES)))
    out = np.concatenate([r["out"] for r in res.results], axis=0)
    return out.reshape(B, C, 32, 32)



# revision 9
# speedup vs baseline: 1.0632x; 1.0632x over previous
"""Trainium2 Bass kernel for nn_AttentionBlock (B=16, C=512, H=W=32, 8 heads).

Sharding: data-parallel over batch across 8 NeuronCores (2 batch elems/core).
No collectives: each core runs the same NEFF on its own batch slice.

Key structure (v2 — overlap-oriented rewrite):
  - All matmuls bf16, N=512, ~219ns/MM measured back-to-back (LDWEIGHTS hides).
  - QKV weights pair-packed so q/k of a head pair land on psum partition
    halves in one [128,1024] chain -> single full-width psum->SBUF copy.
  - Attention: per pair, per j-tile: scores (K=64) -> ScalarE exp (the pacer,
    ~1.1us per [128,1024] ACT); P@V uses the ones-column trick (M=65) for the
    softmax denominator, two sc-passes so its PSUM footprint is 2 banks.
  - Normalization: reciprocal straight from PSUM row 64, GpSimd broadcast,
    DVE multiply into oT.
  - Residual added from the bf16 x copy during the proj psum->SBUF move
    (no DRAM->DRAM precopy, no accumulating DMA).
  - Engine queues are FIFO in emission order, so elem1's QKV chains and
    elem0's proj chains are *emitted inside* elem0/elem1's attention pair
    loops (hooks) to fill PE slack under the ScalarE-paced softmax.
  PSUM budget: scores 2x[128,1024] (4 banks) + PV 2x[65,512] (2 banks)
  + one [128,1024] chain lane (2 banks) = 8 banks.
"""

import numpy as np

import concourse.bacc as bacc
import concourse.bass as bass
import concourse.mybir as mybir
import concourse.tile as tile

F32 = mybir.dt.float32
BF16 = mybir.dt.bfloat16

B, C, HW, NH, DK = 16, 512, 1024, 8, 64
NCORES = 8
BPC = B // NCORES          # batch elems per core
P = 128
KT = C // P                # 4 contraction tiles over C
NPAIR = NH // 2            # 4 head pairs
SC = HW // 512             # 2 s-chunks of 512
ST = HW // P               # 8 s-tiles of 128 (j tiles)
SCALE = DK ** -0.5

MM_DTYPE = "bf16"  # kept for test.py compat; kernel always runs bf16 matmuls


def build_program(with_bias: bool, mm_dtype: str = MM_DTYPE):
    nc = bacc.Bacc(None, target_bir_lowering=False, debug=False)

    x_d = nc.dram_tensor("x", [BPC, C, HW], F32, kind="ExternalInput")
    wqkv_d = nc.dram_tensor("w_qkv", [C, 3 * C], F32, kind="ExternalInput")
    wproj_d = nc.dram_tensor("w_proj", [C, C], F32, kind="ExternalInput")
    if with_bias:
        bqkv_d = nc.dram_tensor("b_qkv", [3 * C], F32, kind="ExternalInput")
        bproj_d = nc.dram_tensor("b_proj", [C], F32, kind="ExternalInput")
    out_d = nc.dram_tensor("out", [BPC, C, HW], F32, kind="ExternalOutput")

    with tile.TileContext(nc) as tc:
        with tc.tile_pool(name="consts", bufs=1) as consts:
            # Pair-packed q/k weights: [kt, pair, {q,k}, (m*64+t)] where the
            # 128 columns of (pair a, qk) are [w_{2a} | w_{2a+1}] head halves.
            wqk_r = consts.tile([P, KT, NPAIR, 2, P], BF16)
            wv_sb = consts.tile([P, KT, C], BF16)
            wproj_r = consts.tile([P, KT, C], BF16)
            warm_i = consts.tile([1, DK], F32)
            warm_o = consts.tile([1, DK], F32)
            if with_bias:
                bq_sb = consts.tile([P, NPAIR], F32)
                bk_sb = consts.tile([P, NPAIR], F32)
                bv_sb = consts.tile([1, C], BF16)
                bp_sb = consts.tile([P, KT], F32)
                ones_f32 = consts.tile([1, P], F32)
                nc.vector.memset(ones_f32, 1.0)
                ones_sb = consts.tile([1, P], BF16)
                nc.vector.tensor_copy(out=ones_sb, in_=ones_f32)

            # Warm the ScalarE exp table set early so the first real ACT
            # doesn't pay the ~1.3us table load.
            nc.vector.memset(warm_i, 1.0)
            nc.scalar.activation(
                out=warm_o, in_=warm_i,
                func=mybir.ActivationFunctionType.Exp, scale=1.0,
            )

            # Weight staging: chunked so the staging pool stays small.
            with tc.tile_pool(name="wstage", bufs=2) as wstagep:
                for kt in range(KT):
                    wst = wstagep.tile([P, 3 * C], F32, tag="wq",
                                       name=f"wst{kt}")
                    eng = nc.scalar if kt % 2 == 0 else nc.gpsimd
                    eng.dma_start(
                        out=wst, in_=wqkv_d[kt * P : (kt + 1) * P, :]
                    )
                    ws4 = wst.rearrange("p (h t) -> p h t", t=3 * DK)
                    for qk in range(2):
                        # pair a's 128 cols = heads (2a, 2a+1) side by side
                        nc.vector.tensor_copy(
                            out=wqk_r[:, kt, :, qk, :].rearrange(
                                "p a (m t) -> p a m t", m=2),
                            in_=ws4[:, :, qk * DK : (qk + 1) * DK].rearrange(
                                "p (a m) t -> p a m t", m=2),
                        )
                    nc.vector.tensor_copy(
                        out=wv_sb[:, kt, :].rearrange("p (h t) -> p h t", t=DK),
                        in_=ws4[:, :, 2 * DK :],
                    )
                for kt in range(KT):
                    wpst = wstagep.tile([P, C], F32, tag="wp",
                                        name=f"wpst{kt}")
                    eng = nc.scalar if kt % 2 == 0 else nc.gpsimd
                    eng.dma_start(
                        out=wpst, in_=wproj_d[kt * P : (kt + 1) * P, :]
                    )
                    nc.vector.tensor_copy(out=wproj_r[:, kt, :], in_=wpst)
                if with_bias:
                    b3 = bqkv_d[:].rearrange("(h t) -> h t", t=3 * DK)
                    for m in range(2):
                        nc.sync.dma_start(
                            out=bq_sb[m * DK : (m + 1) * DK, :],
                            in_=b3[m::2, 0:DK].rearrange("a t -> t a"),
                        )
                        nc.sync.dma_start(
                            out=bk_sb[m * DK : (m + 1) * DK, :],
                            in_=b3[m::2, DK : 2 * DK].rearrange("a t -> t a"),
                        )
                    bv_st = wstagep.tile([1, C], F32, tag="bv")
                    nc.sync.dma_start(
                        out=bv_st,
                        in_=b3[:, 2 * DK :].rearrange("h t -> (h t)")[None, :],
                    )
                    nc.vector.tensor_copy(out=bv_sb, in_=bv_st)
                    nc.sync.dma_start(
                        out=bp_sb, in_=bproj_d[:].rearrange("(a p) -> p a", p=P)
                    )

            with (
                tc.tile_pool(name="xf", bufs=1) as xfp,
                tc.tile_pool(name="xr", bufs=2) as xrp,
                tc.tile_pool(name="qk", bufs=2) as qkp,
                tc.tile_pool(name="vp", bufs=2) as vpp,
                tc.tile_pool(name="pt", bufs=4) as ptp,
                tc.tile_pool(name="ot", bufs=2) as otp,
                tc.tile_pool(name="yt", bufs=2) as ytp,
                tc.tile_pool(name="rc", bufs=2) as rcp,
                tc.tile_pool(name="rcb", bufs=2) as rcbp,
                tc.tile_pool(name="psS", bufs=2, space="PSUM") as psS,
                tc.tile_pool(name="psQ", bufs=1, space="PSUM") as psQ,
                tc.tile_pool(name="psV", bufs=2, space="PSUM") as psV,
            ):
                x_r = [None, None]
                qkT = [None, None]
                v_sb = [None, None]
                oT = [None, None]

                def emit_load(b):
                    """DMA x (f32, in 2 chunks) and cast to bf16 x_r."""
                    x_r[b] = xrp.tile([P, KT, HW], BF16, tag="xr",
                                      name=f"xr{b}")
                    xv = x_d[b].rearrange("(kt p) s -> p kt s", p=P)
                    for ch in range(2):
                        xc = xfp.tile([P, 2, HW], F32, tag="x",
                                      name=f"x{b}_{ch}")
                        nc.sync.dma_start(
                            out=xc, in_=xv[:, 2 * ch : 2 * ch + 2, :]
                        )
                        nc.vector.tensor_copy(
                            out=x_r[b][:, 2 * ch : 2 * ch + 2, :].rearrange(
                                "p k s -> p (k s)"),
                            in_=xc.rearrange("p k s -> p (k s)"),
                        )

                def alloc_attn_bufs(b):
                    qkT[b] = qkp.tile([P, 2, NPAIR, HW], BF16, tag="qkT",
                                      name=f"qkT{b}")
                    v_sb[b] = vpp.tile([P, ST, NH, DK + 1], BF16, tag="v",
                                       name=f"v{b}")
                    oT[b] = otp.tile([P, NPAIR, HW], BF16, tag="oT",
                                     name=f"oT{b}")
                    nc.vector.memset(
                        v_sb[b].rearrange("p st h t -> p (st h) t")[:, :, DK:],
                        1.0,
                    )

                def emit_chain_qk(b, a, qk, pool):
                    """One q-or-k chain for head pair a -> qkT[b][:,qk,a,:]."""
                    ps = pool.tile([P, HW], F32, tag=pool_tag[id(pool)],
                                   name=f"qk{b}_{a}_{qk}")
                    for kt in range(KT):
                        for sc in range(SC):
                            nc.tensor.matmul(
                                ps[:, sc * 512 : (sc + 1) * 512],
                                lhsT=wqk_r[:, kt, a, qk, :],
                                rhs=x_r[b][:, kt, sc * 512 : (sc + 1) * 512],
                                start=(kt == 0),
                                stop=(kt == KT - 1),
                            )
                    if with_bias:
                        bsb = bq_sb if qk == 0 else bk_sb
                        nc.vector.tensor_scalar(
                            out=qkT[b][:, qk, a, :], in0=ps,
                            scalar1=bsb[:, a : a + 1], scalar2=None,
                            op0=mybir.AluOpType.add,
                        )
                    else:
                        nc.vector.tensor_copy(out=qkT[b][:, qk, a, :], in_=ps)

                def emit_chain_v(b, mt2, pool):
                    """v rows for s-tiles 2*mt2, 2*mt2+1 -> v_sb[b]."""
                    ps = pool.tile([P, HW], F32, tag=pool_tag[id(pool)],
                                   name=f"v{b}_{mt2}")
                    for half in range(2):
                        mt = 2 * mt2 + half
                        for kt in range(KT):
                            nc.tensor.matmul(
                                ps[:, half * 512 : (half + 1) * 512],
                                lhsT=x_r[b][:, kt, mt * P : (mt + 1) * P],
                                rhs=wv_sb[:, kt, :],
                                start=(kt == 0),
                                stop=(kt == KT - 1) if not with_bias else False,
                            )
                        if with_bias:
                            nc.tensor.matmul(
                                ps[:, half * 512 : (half + 1) * 512],
                                lhsT=ones_sb, rhs=bv_sb,
                                start=False, stop=True,
                            )
                    nc.vector.tensor_copy(
                        out=v_sb[b][:, 2 * mt2 : 2 * mt2 + 2, :, 0:DK],
                        in_=ps.rearrange("p (k h t) -> p k h t", k=2, h=NH),
                    )

                def emit_chain_proj(b, at):
                    """proj output-channel tile at -> DRAM (with residual)."""
                    ps = psQ.tile([P, HW], F32, tag="q", name=f"pj{b}_{at}")
                    for kt in range(KT):
                        for sc in range(SC):
                            nc.tensor.matmul(
                                ps[:, sc * 512 : (sc + 1) * 512],
                                lhsT=wproj_r[:, kt, at * P : (at + 1) * P],
                                rhs=oT[b][:, kt, sc * 512 : (sc + 1) * 512],
                                start=(kt == 0),
                                stop=(kt == KT - 1),
                            )
                    yt = ytp.tile([P, HW], F32, tag="y", name=f"yt{b}_{at}")
                    if with_bias:
                        nc.vector.scalar_tensor_tensor(
                            out=yt, in0=ps, scalar=bp_sb[:, at : at + 1],
                            in1=x_r[b][:, at, :],
                            op0=mybir.AluOpType.add, op1=mybir.AluOpType.add,
                        )
                    else:
                        nc.vector.tensor_tensor(
                            out=yt, in0=ps, in1=x_r[b][:, at, :],
                            op=mybir.AluOpType.add,
                        )
                    nc.sync.dma_start(
                        out=out_d[b, at * P : (at + 1) * P, :], in_=yt
                    )

                def emit_pv_pass(b_p, a_p, pts_p, p_sc, jj):
                    for m in range(2):
                        nc.tensor.matmul(
                            pv_cur[p_sc % 2][m],
                            lhsT=v_sb[b_p][:, jj, 2 * a_p + m, :],
                            rhs=pts_p[m][:, jj, p_sc * 512 : (p_sc + 1) * 512],
                            start=(jj == 0),
                            stop=(jj == ST - 1),
                        )

                def emit_pv_alloc(b_p, a_p, p_sc):
                    pv_cur[p_sc % 2] = [
                        psV.tile([DK + 1, 512], F32, tag="pv",
                                 name=f"pv{b_p}_{a_p}_{p_sc}_{m}")
                        for m in range(2)
                    ]

                def emit_norm(b_p, a_p, p_sc):
                    for m in range(2):
                        pv = pv_cur[p_sc % 2][m]
                        rs = rcp.tile([1, 512], F32, tag="rs", name="rs")
                        nc.vector.tensor_copy(out=rs, in_=pv[DK : DK + 1, :])
                        rc = rcp.tile([1, 512], F32, tag="rc", name="rc")
                        nc.vector.reciprocal_approx_fast(out=rc, in_=rs)
                        rcb = rcbp.tile([DK, 512], F32, tag="rcb", name="rcb")
                        nc.gpsimd.partition_broadcast(rcb, rc)
                        nc.vector.tensor_tensor(
                            out=oT[b_p][m * DK : (m + 1) * DK, a_p,
                                      p_sc * 512 : (p_sc + 1) * 512],
                            in0=pv[0:DK, :], in1=rcb,
                            op=mybir.AluOpType.mult,
                        )

                def emit_attention(b, hooks, carry=None):
                    """Pair loop; hooks[p] emitted after pair p's jt loop.
                    Pair a's jt loop carries the previous pair's P@V (two sc
                    passes) — including the last pair of the previous elem
                    via `carry` = (b_prev, a_prev, pts_prev). Returns its own
                    last pair as the next carry."""
                    prev = carry
                    for a in range(NPAIR):
                        pts = [
                            ptp.tile([P, ST, HW], BF16, tag="pt",
                                     name=f"pt{b}_{a}_{m}", bufs=4)
                            for m in range(2)
                        ]
                        for jt in range(ST):
                            p_sc = 0 if jt < 4 else 1
                            if prev is not None and jt % 4 == 0:
                                emit_pv_alloc(prev[0], prev[1], p_sc)
                            pss = [
                                psS.tile([P, HW], F32, tag="sc",
                                         name=f"s{b}_{a}_{jt}_{m}")
                                for m in range(2)
                            ]
                            for m in range(2):
                                lo = m * DK
                                for sc in range(SC):
                                    nc.tensor.matmul(
                                        pss[m][:, sc * 512 : (sc + 1) * 512],
                                        lhsT=qkT[b][lo : lo + DK, 1, a,
                                                    jt * P : (jt + 1) * P],
                                        rhs=qkT[b][lo : lo + DK, 0, a,
                                                   sc * 512 : (sc + 1) * 512],
                                        start=True,
                                        stop=True,
                                    )
                            if prev is not None:
                                for half in range(2):
                                    emit_pv_pass(prev[0], prev[1], prev[2],
                                                 p_sc, 2 * (jt % 4) + half)
                            for m in range(2):
                                nc.scalar.activation(
                                    out=pts[m][:, jt, :],
                                    in_=pss[m],
                                    func=mybir.ActivationFunctionType.Exp,
                                    scale=SCALE,
                                )
                            if prev is not None and jt % 4 == 3:
                                emit_norm(prev[0], prev[1], p_sc)
                        for fn in hooks.get(a, []):
                            fn()
                        prev = (b, a, pts)
                    return prev

                def emit_pv_drain(prev):
                    for p_sc in range(SC):
                        emit_pv_alloc(prev[0], prev[1], p_sc)
                        for jj in range(ST):
                            emit_pv_pass(prev[0], prev[1], prev[2], p_sc, jj)
                        emit_norm(prev[0], prev[1], p_sc)

                pool_tag = {id(psS): "sc", id(psQ): "q"}
                pv_cur = [None, None]

                # ---------------- emission schedule ----------------
                # x(0) DMA (sync queue) runs concurrent with weight staging
                # (scalar/gpsimd queues); the barrier funnels all of it.
                emit_load(0)
                tc.strict_bb_all_engine_barrier()
                alloc_attn_bufs(0)
                # preamble: v(0) fully + q/k for pairs 0,1 (3 psum lanes)
                lanes = [psQ, psS, psS]
                for i in range(4):
                    emit_chain_v(0, i, lanes[i % 3])
                for i, (a, qk) in enumerate([(0, 0), (0, 1), (1, 0), (1, 1)]):
                    emit_chain_qk(0, a, qk, lanes[i % 3])
                emit_load(1)
                alloc_attn_bufs(1)

                # attention(0): finish qkv(0) pairs 2-3 early, then qkv(1)
                hooks0 = {
                    0: [lambda: emit_chain_qk(0, 2, 0, psQ),
                        lambda: emit_chain_qk(0, 2, 1, psQ)],
                    1: [lambda: emit_chain_qk(0, 3, 0, psQ),
                        lambda: emit_chain_qk(0, 3, 1, psQ)],
                    2: [lambda: emit_chain_v(1, 0, psQ),
                        lambda: emit_chain_v(1, 1, psQ),
                        lambda: emit_chain_qk(1, 0, 0, psQ)],
                    3: [lambda: emit_chain_v(1, 2, psQ),
                        lambda: emit_chain_v(1, 3, psQ),
                        lambda: emit_chain_qk(1, 0, 1, psQ)],
                }
                carry = emit_attention(0, hooks0)

                # attention(1): remaining qkv(1) chains (lookahead >= 1 pair),
                # then proj(0); elem0's pair-3 P@V rides in via `carry`.
                hooks1 = {
                    0: [lambda: emit_chain_qk(1, 1, 0, psQ),
                        lambda: emit_chain_qk(1, 1, 1, psQ),
                        lambda: emit_chain_qk(1, 2, 0, psQ)],
                    1: [lambda: emit_chain_qk(1, 2, 1, psQ),
                        lambda: emit_chain_qk(1, 3, 0, psQ),
                        lambda: emit_chain_qk(1, 3, 1, psQ)],
                    2: [lambda: emit_chain_proj(0, 0),
                        lambda: emit_chain_proj(0, 1)],
                    3: [lambda: emit_chain_proj(0, 2),
                        lambda: emit_chain_proj(0, 3)],
                }
                carry = emit_attention(1, hooks1, carry)

                emit_pv_drain(carry)
                for at in range(KT):
                    emit_chain_proj(1, at)

    nc.finalize()
    return nc


_CACHE = {}


def _get_program(with_bias: bool, mm_dtype: str = MM_DTYPE):
    key = (with_bias,)
    if key not in _CACHE:
        _CACHE[key] = build_program(with_bias, mm_dtype)
    return _CACHE[key]


def kernel(x, w_qkv, b_qkv, w_proj, b_proj):
    x = np.ascontiguousarray(np.asarray(x, dtype=np.float32)).reshape(B, C, HW)
    w_qkv = np.ascontiguousarray(np.asarray(w_qkv, dtype=np.float32))
    b_qkv = np.ascontiguousarray(np.asarray(b_qkv, dtype=np.float32))
    w_proj = np.ascontiguousarray(np.asarray(w_proj, dtype=np.float32))
    b_proj = np.ascontiguousarray(np.asarray(b_proj, dtype=np.float32))

    with_bias = bool(np.any(b_qkv) or np.any(b_proj))
    nc = _get_program(with_bias, MM_DTYPE)

    in_maps = []
    for i in range(NCORES):
        m = {
            "x": x[i * BPC : (i + 1) * BPC],
            "w_qkv": w_qkv,
            "w_proj": w_proj,
        }
        if with_bias:
            m["b_qkv"] = b_qkv
            m["b_proj"] = b_proj
        in_maps.append(m)

    from concourse.bass_utils import run_bass_kernel_spmd

    res = run_bass_kernel_spmd(nc, in_maps, core_ids=list(range(NCORES)))
    out = np.concatenate([r["out"] for r in res.results], axis=0)
    return out.reshape(B, C, 32, 32)


# revision 20
# speedup vs baseline: 1.3837x; 1.3015x over previous
"""Trainium2 Bass kernel for nn_AttentionBlock (B=16, C=512, H=W=32, 8 heads).

Sharding: data-parallel over batch across 8 NeuronCores (2 batch elems/core).
No collectives: each core runs the same NEFF on its own batch slice.

Key structure (v2 — overlap-oriented rewrite):
  - All matmuls bf16, N=512, ~219ns/MM measured back-to-back (LDWEIGHTS hides).
  - QKV weights pair-packed so q/k of a head pair land on psum partition
    halves in one [128,1024] chain -> single full-width psum->SBUF copy.
  - Attention: per pair, per j-tile: scores (K=64) -> ScalarE exp (the pacer,
    ~1.1us per [128,1024] ACT); P@V uses the ones-column trick (M=65) for the
    softmax denominator, two sc-passes so its PSUM footprint is 2 banks.
  - Normalization: reciprocal straight from PSUM row 64, GpSimd broadcast,
    DVE multiply into oT.
  - Residual added from the bf16 x copy during the proj psum->SBUF move
    (no DRAM->DRAM precopy, no accumulating DMA).
  - Engine queues are FIFO in emission order, so elem1's QKV chains and
    elem0's proj chains are *emitted inside* elem0/elem1's attention pair
    loops (hooks) to fill PE slack under the ScalarE-paced softmax.
  PSUM budget: scores 2x[128,1024] (4 banks) + PV 2x[65,512] (2 banks)
  + one [128,1024] chain lane (2 banks) = 8 banks.
"""

import numpy as np

import concourse.bacc as bacc
import concourse.bass as bass
import concourse.mybir as mybir
import concourse.tile as tile

F32 = mybir.dt.float32
BF16 = mybir.dt.bfloat16

B, C, HW, NH, DK = 16, 512, 1024, 8, 64
NCORES = 8
BPC = B // NCORES          # batch elems per core
P = 128
KT = C // P                # 4 contraction tiles over C
NPAIR = NH // 2            # 4 head pairs
SC = HW // 512             # 2 s-chunks of 512
ST = HW // P               # 8 s-tiles of 128 (j tiles)
SCALE = DK ** -0.5

MM_DTYPE = "bf16"  # kept for test.py compat; kernel always runs bf16 matmuls


def build_program(with_bias: bool, mm_dtype: str = MM_DTYPE):
    nc = bacc.Bacc(None, target_bir_lowering=False, debug=False)

    x_d = nc.dram_tensor("x", [BPC, C, HW], F32, kind="ExternalInput")
    wqkv_d = nc.dram_tensor("w_qkv", [C, 3 * C], F32, kind="ExternalInput")
    wproj_d = nc.dram_tensor("w_proj", [C, C], F32, kind="ExternalInput")
    if with_bias:
        bqkv_d = nc.dram_tensor("b_qkv", [3 * C], F32, kind="ExternalInput")
        bproj_d = nc.dram_tensor("b_proj", [C], F32, kind="ExternalInput")
    out_d = nc.dram_tensor("out", [BPC, C, HW], BF16,
                           kind="ExternalOutput")

    with tile.TileContext(nc) as tc:
        with tc.tile_pool(name="consts", bufs=1) as consts:
            # Pair-packed q/k weights: [kt, pair, {q,k}, (m*64+t)] where the
            # 128 columns of (pair a, qk) are [w_{2a} | w_{2a+1}] head halves.
            wqk_r = consts.tile([P, KT, NPAIR, 2, P], BF16)
            wv_sb = consts.tile([P, KT, C], BF16)
            wproj_r = consts.tile([P, KT, C], BF16)
            warm_i = consts.tile([1, DK], F32)
            warm_o = consts.tile([1, DK], F32)
            # Zero-padded q: per (pair, m) the rhs is [128, S] with q_h on
            # partition half m and ZEROS on the other half, so score matmuls
            # run K=128 (no 64-row tiling mode switch, which drains the PE).
            qzp = [consts.tile([P, NPAIR, 2, HW], BF16, name=f"qzp{b}")
                   for b in range(BPC)]
            if with_bias:
                bq_sb = consts.tile([P, NPAIR], F32)
                bk_sb = consts.tile([P, NPAIR], F32)
                bv_sb = consts.tile([1, C], BF16)
                bp_sb = consts.tile([P, KT], F32)
                ones_f32 = consts.tile([1, P], F32)
                nc.vector.memset(ones_f32, 1.0)
                ones_sb = consts.tile([1, P], BF16)
                nc.vector.tensor_copy(out=ones_sb, in_=ones_f32)

            nc.vector.memset(warm_i, 1.0)
            # half-masks: qzp is written as q * mask so its zero half never
            # needs a bulk memset
            hmask = consts.tile([P, 2], F32)
            nc.vector.memset(hmask, 0.0)
            nc.vector.memset(hmask[0:DK, 0:1], 1.0)
            nc.vector.memset(hmask[DK:P, 1:2], 1.0)

            # Weight staging: all wqkv chunks issued up front on the
            # scalar HWDGE queue (parallel rings); x rides the sync queue;
            # wproj (not needed until proj(0)) goes to the slow gpsimd SWDGE.
            with tc.tile_pool(name="wstage", bufs=4) as wstagep:
                wsts = []
                for kt in range(KT):
                    wst = wstagep.tile([P, 3 * C], F32, tag="wq",
                                       name=f"wst{kt}", bufs=4)
                    nc.scalar.dma_start(
                        out=wst, in_=wqkv_d[kt * P : (kt + 1) * P, :]
                    )
                    wsts.append(wst)
                # Warm the ScalarE exp table set (after the DMA triggers so
                # it doesn't delay them; before any real ACT).
                nc.scalar.activation(
                    out=warm_o, in_=warm_i,
                    func=mybir.ActivationFunctionType.Exp, scale=1.0,
                )
                for kt in range(KT):
                    ws4 = wsts[kt].rearrange("p (h t) -> p h t", t=3 * DK)
                    for qk in range(2):
                        # pair a's 128 cols = heads (2a, 2a+1) side by side
                        nc.vector.tensor_copy(
                            out=wqk_r[:, kt, :, qk, :].rearrange(
                                "p a (m t) -> p a m t", m=2),
                            in_=ws4[:, :, qk * DK : (qk + 1) * DK].rearrange(
                                "p (a m) t -> p a m t", m=2),
                        )
                    nc.vector.tensor_copy(
                        out=wv_sb[:, kt, :].rearrange("p (h t) -> p h t", t=DK),
                        in_=ws4[:, :, 2 * DK :],
                    )
                if with_bias:
                    b3 = bqkv_d[:].rearrange("(h t) -> h t", t=3 * DK)
                    for m in range(2):
                        nc.sync.dma_start(
                            out=bq_sb[m * DK : (m + 1) * DK, :],
                            in_=b3[m::2, 0:DK].rearrange("a t -> t a"),
                        )
                        nc.sync.dma_start(
                            out=bk_sb[m * DK : (m + 1) * DK, :],
                            in_=b3[m::2, DK : 2 * DK].rearrange("a t -> t a"),
                        )
                    bv_st = wstagep.tile([1, C], F32, tag="bv")
                    nc.sync.dma_start(
                        out=bv_st,
                        in_=b3[:, 2 * DK :].rearrange("h t -> (h t)")[None, :],
                    )
                    nc.vector.tensor_copy(out=bv_sb, in_=bv_st)
                    nc.sync.dma_start(
                        out=bp_sb, in_=bproj_d[:].rearrange("(a p) -> p a", p=P)
                    )

            with (
                tc.tile_pool(name="xf", bufs=1) as xfp,
                tc.tile_pool(name="xr", bufs=2) as xrp,
                tc.tile_pool(name="qk", bufs=2) as qkp,
                tc.tile_pool(name="vp", bufs=2) as vpp,
                tc.tile_pool(name="pt", bufs=4) as ptp,
                tc.tile_pool(name="ot", bufs=2) as otp,
                tc.tile_pool(name="yt", bufs=1) as ytp,
                tc.tile_pool(name="rc", bufs=2) as rcp,
                tc.tile_pool(name="rcb", bufs=2) as rcbp,
                tc.tile_pool(name="psS", bufs=2, space="PSUM") as psS,
                tc.tile_pool(name="psQ", bufs=1, space="PSUM") as psQ,
                tc.tile_pool(name="psV", bufs=2, space="PSUM") as psV,
            ):
                x_r = [None, None]
                kT = [None, None]
                v_sb = [None, None]
                oT = [None, None]

                def emit_load(b):
                    """DMA x (f32, in 2 chunks) and cast to bf16 x_r."""
                    x_r[b] = xrp.tile([P, KT, HW], BF16, tag="xr",
                                      name=f"xr{b}")
                    xv = x_d[b].rearrange("(kt p) s -> p kt s", p=P)
                    for ch in range(2):
                        xc = xfp.tile([P, 2, HW], F32, tag="x",
                                      name=f"x{b}_{ch}")
                        nc.sync.dma_start(
                            out=xc, in_=xv[:, 2 * ch : 2 * ch + 2, :]
                        )
                        nc.vector.tensor_copy(
                            out=x_r[b][:, 2 * ch : 2 * ch + 2, :].rearrange(
                                "p k s -> p (k s)"),
                            in_=xc.rearrange("p k s -> p (k s)"),
                        )

                def alloc_attn_bufs(b):
                    kT[b] = qkp.tile([P, NPAIR, HW], BF16, tag="kT",
                                     name=f"kT{b}")
                    v_sb[b] = vpp.tile([P, ST, NH, DK + 1], BF16, tag="v",
                                       name=f"v{b}")
                    oT[b] = otp.tile([P, NPAIR, HW], BF16, tag="oT",
                                     name=f"oT{b}")
                    nc.vector.memset(
                        v_sb[b].rearrange("p st h t -> p (st h) t")[:, :, DK:],
                        1.0,
                    )

                def emit_chain_qk(b, a, qk, pool):
                    """One q-or-k chain for head pair a -> qkT[b][:,qk,a,:]."""
                    ps = pool.tile([P, HW], F32, tag=pool_tag[id(pool)],
                                   name=f"qk{b}_{a}_{qk}")
                    for kt in range(KT):
                        for sc in range(SC):
                            nc.tensor.matmul(
                                ps[:, sc * 512 : (sc + 1) * 512],
                                lhsT=wqk_r[:, kt, a, qk, :],
                                rhs=x_r[b][:, kt, sc * 512 : (sc + 1) * 512],
                                start=(kt == 0),
                                stop=(kt == KT - 1),
                            )
                    if qk == 0:
                        for m in range(2):
                            if with_bias:
                                nc.vector.tensor_scalar(
                                    out=qzp[b][:, a, m, :], in0=ps,
                                    scalar1=bq_sb[:, a : a + 1],
                                    scalar2=hmask[:, m : m + 1],
                                    op0=mybir.AluOpType.add,
                                    op1=mybir.AluOpType.mult,
                                )
                            else:
                                nc.vector.tensor_scalar(
                                    out=qzp[b][:, a, m, :], in0=ps,
                                    scalar1=hmask[:, m : m + 1],
                                    scalar2=None,
                                    op0=mybir.AluOpType.mult,
                                )
                    elif with_bias:
                        nc.vector.tensor_scalar(
                            out=kT[b][:, a, :], in0=ps,
                            scalar1=bk_sb[:, a : a + 1], scalar2=None,
                            op0=mybir.AluOpType.add,
                        )
                    else:
                        nc.vector.tensor_copy(out=kT[b][:, a, :], in_=ps)

                def emit_chain_v(b, mt2, pool):
                    """v rows for s-tiles 2*mt2, 2*mt2+1 -> v_sb[b]."""
                    ps = pool.tile([P, HW], F32, tag=pool_tag[id(pool)],
                                   name=f"v{b}_{mt2}")
                    for half in range(2):
                        mt = 2 * mt2 + half
                        for kt in range(KT):
                            nc.tensor.matmul(
                                ps[:, half * 512 : (half + 1) * 512],
                                lhsT=x_r[b][:, kt, mt * P : (mt + 1) * P],
                                rhs=wv_sb[:, kt, :],
                                start=(kt == 0),
                                stop=(kt == KT - 1) if not with_bias else False,
                            )
                        if with_bias:
                            nc.tensor.matmul(
                                ps[:, half * 512 : (half + 1) * 512],
                                lhsT=ones_sb, rhs=bv_sb,
                                start=False, stop=True,
                            )
                    nc.vector.tensor_copy(
                        out=v_sb[b][:, 2 * mt2 : 2 * mt2 + 2, :, 0:DK],
                        in_=ps.rearrange("p (k h t) -> p k h t", k=2, h=NH),
                    )

                def emit_chain_proj(b, at, pool=None):
                    """proj output-channel tile at -> DRAM (with residual)."""
                    pool = pool if pool is not None else psQ
                    ps = pool.tile([P, HW], F32, tag=pool_tag[id(pool)],
                                   name=f"pj{b}_{at}")
                    for kt in range(KT):
                        for sc in range(SC):
                            nc.tensor.matmul(
                                ps[:, sc * 512 : (sc + 1) * 512],
                                lhsT=wproj_r[:, kt, at * P : (at + 1) * P],
                                rhs=oT[b][:, kt, sc * 512 : (sc + 1) * 512],
                                start=(kt == 0),
                                stop=(kt == KT - 1),
                            )
                    yt = ytp.tile([P, HW], BF16, tag="y", name=f"yt{b}_{at}")
                    if with_bias:
                        nc.vector.scalar_tensor_tensor(
                            out=yt, in0=ps, scalar=bp_sb[:, at : at + 1],
                            in1=x_r[b][:, at, :],
                            op0=mybir.AluOpType.add, op1=mybir.AluOpType.add,
                        )
                    else:
                        nc.vector.tensor_tensor(
                            out=yt, in0=ps, in1=x_r[b][:, at, :],
                            op=mybir.AluOpType.add,
                        )
                    deng = nc.sync if at % 2 == 0 else nc.scalar
                    deng.dma_start(
                        out=out_d[b, at * P : (at + 1) * P, :], in_=yt
                    )

                def emit_pv_pass(b_p, a_p, pts_p, p_sc, jj):
                    for m in range(2):
                        nc.tensor.matmul(
                            pv_cur[p_sc % 2][m],
                            lhsT=v_sb[b_p][:, jj, 2 * a_p + m, :],
                            rhs=pts_p[m][:, jj, p_sc * 512 : (p_sc + 1) * 512],
                            start=(jj == 0),
                            stop=(jj == ST - 1),
                        )

                def emit_pv_alloc(b_p, a_p, p_sc):
                    pv_cur[p_sc % 2] = [
                        psV.tile([DK + 1, 512], F32, tag="pv",
                                 name=f"pv{b_p}_{a_p}_{p_sc}_{m}")
                        for m in range(2)
                    ]

                def emit_norm(b_p, a_p, p_sc):
                    for m in range(2):
                        pv = pv_cur[p_sc % 2][m]
                        rs = rcp.tile([1, 512], F32, tag="rs", name="rs")
                        nc.vector.tensor_copy(out=rs, in_=pv[DK : DK + 1, :])
                        rc = rcp.tile([1, 512], F32, tag="rc", name="rc")
                        nc.vector.reciprocal_approx_fast(out=rc, in_=rs)
                        rcb = rcbp.tile([DK, 512], F32, tag="rcb", name="rcb")
                        nc.gpsimd.partition_broadcast(rcb, rc)
                        nc.vector.tensor_tensor(
                            out=oT[b_p][m * DK : (m + 1) * DK, a_p,
                                      p_sc * 512 : (p_sc + 1) * 512],
                            in0=pv[0:DK, :], in1=rcb,
                            op=mybir.AluOpType.mult,
                        )

                def emit_attention(b, hooks, carry=None):
                    """Pair loop; hooks[p] emitted after pair p's jt loop.
                    Pair a's jt loop carries the previous pair's P@V (two sc
                    passes) — including the last pair of the previous elem
                    via `carry` = (b_prev, a_prev, pts_prev). Returns its own
                    last pair as the next carry."""
                    prev = carry
                    for a in range(NPAIR):
                        chain_q = list(hooks.get(a, []))
                        pts = [
                            ptp.tile([P, ST, HW], BF16, tag="pt",
                                     name=f"pt{b}_{a}_{m}", bufs=4)
                            for m in range(2)
                        ]
                        for jt in range(ST):
                            p_sc = 0 if jt < 4 else 1
                            if prev is not None and jt % 4 == 0:
                                emit_pv_alloc(prev[0], prev[1], p_sc)
                            pss = [
                                psS.tile([P, HW], F32, tag="sc",
                                         name=f"s{b}_{a}_{jt}_{m}")
                                for m in range(2)
                            ]
                            for sc in range(SC):
                                for m in range(2):
                                    nc.tensor.matmul(
                                        pss[m][:, sc * 512 : (sc + 1) * 512],
                                        lhsT=kT[b][:, a,
                                                   jt * P : (jt + 1) * P],
                                        rhs=qzp[b][:, a, m,
                                                   sc * 512 : (sc + 1) * 512],
                                        start=True,
                                        stop=True,
                                    )
                            if prev is not None:
                                for half in range(2):
                                    emit_pv_pass(prev[0], prev[1], prev[2],
                                                 p_sc, 2 * (jt % 4) + half)
                            for m in range(2):
                                nc.scalar.activation(
                                    out=pts[m][:, jt, :],
                                    in_=pss[m],
                                    func=mybir.ActivationFunctionType.Exp,
                                    scale=SCALE,
                                )
                            if prev is not None and jt % 4 == 3:
                                emit_norm(prev[0], prev[1], p_sc)
                            if jt in (1, 3, 5) and chain_q:
                                chain_q.pop(0)()
                        prev = (b, a, pts)
                    return prev

                def emit_pv_drain(prev):
                    for p_sc in range(SC):
                        emit_pv_alloc(prev[0], prev[1], p_sc)
                        for jj in range(ST):
                            emit_pv_pass(prev[0], prev[1], prev[2], p_sc, jj)
                        emit_norm(prev[0], prev[1], p_sc)

                pool_tag = {id(psS): "sc", id(psQ): "q"}
                pv_cur = [None, None]

                # ---------------- emission schedule ----------------
                # x(0) DMA (sync queue) runs concurrent with weight staging
                # (scalar/gpsimd queues); the barrier funnels all of it.
                emit_load(0)
                with tc.tile_pool(name="wpstage", bufs=2) as wpstagep:
                    for kt in range(KT):
                        wpst = wpstagep.tile([P, C], F32, tag="wp",
                                             name=f"wpst{kt}")
                        nc.gpsimd.dma_start(
                            out=wpst, in_=wproj_d[kt * P : (kt + 1) * P, :]
                        )
                        nc.gpsimd.tensor_copy(out=wproj_r[:, kt, :], in_=wpst)
                alloc_attn_bufs(0)
                # preamble: only q/k for pairs 0,1 (pair-0 scores gate on
                # them; keep the psS FIFO short). v(0) chains ride pair-0's
                # hook slots — P@V first touches them during pair 1.
                for i, (a, qk, pool) in enumerate(
                    [(0, 0, psQ), (0, 1, psS), (1, 0, psS), (1, 1, psQ)]
                ):
                    emit_chain_qk(0, a, qk, pool)
                emit_load(1)
                alloc_attn_bufs(1)

                # attention(0): finish qkv(0) pairs 2-3 early, then qkv(1)
                hooks0 = {
                    0: [lambda: emit_chain_v(0, 0, psQ),
                        lambda: emit_chain_v(0, 1, psQ),
                        lambda: emit_chain_v(0, 2, psQ)],
                    1: [lambda: emit_chain_v(0, 3, psQ),
                        lambda: emit_chain_qk(0, 2, 0, psQ),
                        lambda: emit_chain_qk(0, 2, 1, psQ)],
                    2: [lambda: emit_chain_qk(0, 3, 0, psQ),
                        lambda: emit_chain_qk(0, 3, 1, psQ),
                        lambda: emit_chain_v(1, 0, psQ)],
                    3: [lambda: emit_chain_v(1, 1, psQ),
                        lambda: emit_chain_qk(1, 0, 0, psQ),
                        lambda: emit_chain_qk(1, 0, 1, psQ)],
                }
                carry = emit_attention(0, hooks0)

                # attention(1): remaining qkv(1) chains (>=1 pair lookahead),
                # then proj(0); elem0's pair-3 P@V rides in via `carry`.
                hooks1 = {
                    0: [lambda: emit_chain_qk(1, 1, 0, psQ),
                        lambda: emit_chain_qk(1, 1, 1, psQ),
                        lambda: emit_chain_v(1, 2, psQ)],
                    1: [lambda: emit_chain_v(1, 3, psQ),
                        lambda: emit_chain_qk(1, 2, 0, psQ),
                        lambda: emit_chain_qk(1, 2, 1, psQ)],
                    2: [lambda: emit_chain_qk(1, 3, 0, psQ),
                        lambda: emit_chain_qk(1, 3, 1, psQ),
                        lambda: emit_chain_proj(0, 0)],
                    3: [lambda: emit_chain_proj(0, 1),
                        lambda: emit_chain_proj(0, 2),
                        lambda: emit_chain_proj(0, 3)],
                }
                carry = emit_attention(1, hooks1, carry)

                emit_pv_drain(carry)
                lanes = [psQ, psS, psS]
                for at in range(KT):
                    emit_chain_proj(1, at, lanes[at % 3])

    nc.finalize()
    return nc


_CACHE = {}


def _get_program(with_bias: bool, mm_dtype: str = MM_DTYPE):
    key = (with_bias,)
    if key not in _CACHE:
        _CACHE[key] = build_program(with_bias, mm_dtype)
    return _CACHE[key]


def kernel(x, w_qkv, b_qkv, w_proj, b_proj):
    x = np.ascontiguousarray(np.asarray(x, dtype=np.float32)).reshape(B, C, HW)
    w_qkv = np.ascontiguousarray(np.asarray(w_qkv, dtype=np.float32))
    b_qkv = np.ascontiguousarray(np.asarray(b_qkv, dtype=np.float32))
    w_proj = np.ascontiguousarray(np.asarray(w_proj, dtype=np.float32))
    b_proj = np.ascontiguousarray(np.asarray(b_proj, dtype=np.float32))

    with_bias = bool(np.any(b_qkv) or np.any(b_proj))
    nc = _get_program(with_bias, MM_DTYPE)

    in_maps = []
    for i in range(NCORES):
        m = {
            "x": x[i * BPC : (i + 1) * BPC],
            "w_qkv": w_qkv,
            "w_proj": w_proj,
        }
        if with_bias:
            m["b_qkv"] = b_qkv
            m["b_proj"] = b_proj
        in_maps.append(m)

    from concourse.bass_utils import run_bass_kernel_spmd

    res = run_bass_kernel_spmd(nc, in_maps, core_ids=list(range(NCORES)))
    out = np.concatenate(
        [np.asarray(r["out"], dtype=np.float32) for r in res.results], axis=0
    )
    return out.reshape(B, C, 32, 32)
